# revision 16
# baseline (speedup 1.0000x reference)
"""Multi-head linear attention (elu+1 feature map) on 8 TRN2 NeuronCores.

Sharding: core c handles batch b = c//2, sequence half j = c%2 (2048 rows).
Each core computes q/k/v projections + phi + partial kv/z for its rows,
AllReduces kv/z across the (b, j) pair, then computes num/den/ctx and the
output projection for its rows. All matmuls in bf16 (fp32 PSUM accumulate).

This version is built around keeping the PE continuously busy (it ramps
from 1.2GHz to 2.4GHz only after ~3us of uninterrupted work):
  - software-pipelined slots: pair p's projections run while pair p-1's
    kv accumulates, pair p-2's den and pair p-3's num consume the
    AllReduce results of their own per-pair collective (8 small
    collectives pipeline behind compute instead of one late barrier).
  - phi(x) = elu(x)+1 = max(x+1, min(exp(x), 1)) exactly: Exp on the
    Act engine, the clamp min(E,1) on GPSIMD (SBUF-only), and a fused
    scalar_tensor_tensor (add 1 then max) on DVE straight out of PSUM.
  - kf|v computed by one 256-wide matmul per 128-row chunk; z rides the
    kv accumulation as a ones-column of v.
  - den for all 4 s-quarters packs into one PSUM bank via tile_position
    quadrants, so the Act reciprocal pass costs 512 cols per pair; the
    reciprocal broadcast is 2 DMAs per pair via partition_broadcast.
"""
import numpy as np
import ml_dtypes
import os

DEBUG = bool(os.environ.get("KDEBUG"))

B, S, H, Dh = 4, 4096, 16, 64
E = H * Dh
N_CORES = 8
SL = S // 2          # sequence rows per core
NPAIR = H // 2       # head pairs
EPS = 1e-6

_CACHE = {}


def _build_program():
    import concourse.bacc as bacc
    import concourse.mybir as mybir
    import concourse.tile as tile

    bf16 = mybir.dt.bfloat16
    f32 = mybir.dt.float32
    Act = mybir.ActivationFunctionType
    Alu = mybir.AluOpType

    nc = bacc.Bacc(None, target_bir_lowering=False, num_devices=N_CORES)

    xq = nc.dram_tensor("xqT", [E, SL], bf16, kind="ExternalInput")
    wq_bd = nc.dram_tensor("wq_bd", [NPAIR, 128, 128], bf16, kind="ExternalInput")
    wkv_bd = nc.dram_tensor("wkv_bd", [NPAIR, 128, 256], bf16, kind="ExternalInput")
    wo = nc.dram_tensor("wo", [E, E], bf16, kind="ExternalInput")
    y = nc.dram_tensor("y", [SL, E], f32, kind="ExternalOutput")
    NCHUNK = SL // 128   # 128-row s-chunks per pair (16)
    NQC = SL // 512      # 512-wide chunks for qf / num / den (4)
    if DEBUG:
        bf = mybir.dt.bfloat16
        dbg_qf = nc.dram_tensor("dbg_qf", [128, NPAIR, SL], bf, kind="ExternalOutput")
        dbg_ctx = nc.dram_tensor("dbg_ctx", [128, NPAIR, SL], bf, kind="ExternalOutput")
        dbg_kvrd = nc.dram_tensor("dbg_kvrd", [128, NPAIR, 129], bf, kind="ExternalOutput")
        dbg_kf = nc.dram_tensor("dbg_kf", [128, NCHUNK, 128], bf, kind="ExternalOutput")
        dbg_rec = nc.dram_tensor("dbg_rec", [128, NPAIR, SL], bf, kind="ExternalOutput")
        dbg_eq = nc.dram_tensor("dbg_eq", [128, NQC, 512], bf, kind="ExternalOutput")
        dbg_qps = nc.dram_tensor("dbg_qps", [128, NQC, 512], f32, kind="ExternalOutput")
    kv_ar = [
        nc.dram_tensor(f"kv_ar{p}", [128, 129], bf16) for p in range(NPAIR)
    ]

    groups = [[0, 1], [2, 3], [4, 5], [6, 7]]

    with tile.TileContext(nc) as tc:
        with (
            tc.tile_pool(name="persist", bufs=1) as persist,
            tc.tile_pool(name="xp", bufs=2) as xp,
            tc.tile_pool(name="kvsb", bufs=2) as kvsb,
            tc.tile_pool(name="tmp", bufs=2) as tmp,
            tc.tile_pool(name="bdp", bufs=2) as bdp,
            tc.tile_pool(name="rbcp", bufs=2) as rbcp,
            tc.tile_pool(name="outp", bufs=2) as outp,
            tc.tile_pool(name="dram", bufs=1, space="DRAM") as dram,
        ):
            # ---- weights / inputs ----
            wkv_sb = persist.tile([128, NPAIR, 256], bf16)
            nc.sync.dma_start(out=wkv_sb[:], in_=wkv_bd.rearrange("p k m -> k p m"))
            xTs = []
            for p in range(NPAIR):
                xTs.append(
                    xp.tile([128, SL], bf16, tag=f"xT{p}", name=f"xT{p}")
                )
            for p in range(3):
                nc.sync.dma_start(out=xTs[p][:], in_=xq[p * 128:(p + 1) * 128, :])
            wq_sb = persist.tile([128, NPAIR, 128], bf16)
            nc.sync.dma_start(out=wq_sb[:], in_=wq_bd.rearrange("p k m -> k p m"))
            wo_sb = persist.tile([128, NPAIR, E], bf16)
            nc.gpsimd.dma_start(
                out=wo_sb[:], in_=wo.rearrange("(k p) n -> p k n", p=128)
            )
            qfT = persist.tile([128, NPAIR, SL], bf16)
            ctxT = persist.tile([128, NPAIR, SL], bf16)
            kvrd = persist.tile([128, NPAIR, 129], bf16)
            eps_sb = persist.tile([98, 1], f32)
            nc.vector.memset(eps_sb[:], EPS)
            kv_in = [
                dram.tile([128, 129], bf16, tag=f"kvin{p}", name=f"kvin{p}")
                for p in range(NPAIR)
            ]
            rec_dr = [
                dram.tile([98, 512], bf16, tag=f"rec{p}", name=f"rec{p}")
                for p in range(NPAIR)
            ]

            import contextlib

            psA = contextlib.ExitStack()
            ps_kvp = psA.enter_context(
                tc.tile_pool(name="ps_kvp", bufs=2, space="PSUM")
            )
            ps_qf = psA.enter_context(tc.tile_pool(name="ps_qf", bufs=2, space="PSUM"))
            ps_kv = psA.enter_context(tc.tile_pool(name="ps_kv", bufs=1, space="PSUM"))
            ps_den = psA.enter_context(
                tc.tile_pool(name="ps_den", bufs=1, space="PSUM")
            )
            ps_num = psA.enter_context(
                tc.tile_pool(name="ps_num", bufs=2, space="PSUM")
            )

            def emit_proj(p):
                # kf|v projections, s-major: one 256-wide matmul per chunk.
                if p + 3 < NPAIR:
                    q = p + 3
                    nc.sync.dma_start(
                        out=xTs[q][:], in_=xq[q * 128:(q + 1) * 128, :]
                    )
                xT = xTs[p]
                kf = kvsb.tile([128, NCHUNK, 128], bf16, tag="kf")
                vsb = kvsb.tile([128, NCHUNK, 132], bf16, tag="v")
                nc.gpsimd.memset(vsb[:, :, 128:129], 1.0)
                for g in range(NCHUNK // 2):
                    kvps = ps_kvp.tile([128, 2, 256], f32, tag="kvps")
                    for c2 in range(2):
                        i = 2 * g + c2
                        nc.tensor.matmul(
                            kvps[:, c2, :],
                            lhsT=xT[:, i * 128:(i + 1) * 128],
                            rhs=wkv_sb[:, p, :],
                            start=True, stop=True,
                        )
                    cs = slice(2 * g, 2 * g + 2)
                    ek = tmp.tile([128, 2, 128], bf16, tag="ek")
                    nc.scalar.activation(ek[:], kvps[:, :, 0:128], Act.Exp)
                    nc.scalar.copy(vsb[:, cs, 0:128], kvps[:, :, 128:256])
                    nc.gpsimd.tensor_scalar_min(ek[:], ek[:], 1.0)
                    nc.vector.scalar_tensor_tensor(
                        kf[:, cs, :], kvps[:, :, 0:128], 1.0, ek[:],
                        Alu.add, Alu.max,
                    )
                return kf, vsb

            def emit_qf(p):
                xT = xTs[p]
                for j in range(NQC):
                    js = slice(j * 512, (j + 1) * 512)
                    qps = ps_qf.tile([128, 512], f32, tag="qps")
                    nc.tensor.matmul(
                        qps[:], lhsT=wq_sb[:, p, :], rhs=xT[:, js],
                        start=True, stop=True,
                    )
                    eq = tmp.tile([128, 512], bf16, tag="eq")
                    nc.scalar.activation(eq[:], qps[:], Act.Exp)
                    nc.gpsimd.tensor_scalar_min(eq[:], eq[:], 1.0)
                    if DEBUG and p == 0:
                        nc.sync.dma_start(out=dbg_eq[:, j, :], in_=eq[:])
                        dqp = tmp.tile([128, 512], f32, tag="dqp", name="dqp")
                        nc.vector.tensor_copy(dqp[:], qps[:])
                        nc.sync.dma_start(out=dbg_qps[:, j, :], in_=dqp[:])
                    nc.vector.scalar_tensor_tensor(
                        qfT[:, p, js], qps[:], 1.0, eq[:], Alu.add, Alu.max
                    )

            def emit_kv(p, kf, vsb):
                if DEBUG and p == 6:
                    nc.sync.dma_start(out=dbg_kf[:], in_=kf[:])
                kvacc = ps_kv.tile([128, 129], f32, tag="kvacc")
                for i in range(NCHUNK):
                    nc.tensor.matmul(
                        kvacc[:],
                        lhsT=kf[:, i, :], rhs=vsb[:, i, 0:129],
                        start=(i == 0), stop=(i == NCHUNK - 1),
                    )
                kvst = outp.tile([128, 129], bf16, tag="kvst")
                nc.scalar.copy(kvst[:], kvacc[:])
                nc.sync.dma_start(out=kv_in[p][:], in_=kvst[:])
                nc.gpsimd.collective_compute(
                    "AllReduce", mybir.AluOpType.add, replica_groups=groups,
                    ins=[kv_in[p][:]], outs=[kv_ar[p][:]],
                )
                nc.sync.dma_start(out=kvrd[:, p, :], in_=kv_ar[p][:])

            def emit_den(p):
                # z columns zero-padded per head; same stationary reused for
                # all 4 s-quarters via tile_position output quadrants.
                zbd = bdp.tile([128, 2], bf16, tag="zbd")
                nc.gpsimd.memset(zbd[:], 0.0)
                nc.gpsimd.tensor_copy(zbd[0:64, 0:1], kvrd[0:64, p, 128:129])
                nc.gpsimd.tensor_copy(zbd[64:128, 1:2], kvrd[64:128, p, 128:129])
                denps = ps_den.tile([98, 512], f32, tag="denps")
                for k in range(NQC):
                    nc.tensor.matmul(
                        denps[32 * k:32 * k + 2, :],
                        lhsT=zbd[:], rhs=qfT[:, p, k * 512:(k + 1) * 512],
                        start=True, stop=True,
                        tile_position=(0, 32 * k),
                    )
                lnd = tmp.tile([98, 512], f32, tag="lnd")
                nc.scalar.activation(lnd[:], denps[:], Act.Ln, bias=eps_sb[:])
                recb = tmp.tile([98, 512], bf16, tag="recb")
                nc.scalar.activation(recb[:], lnd[:], Act.Exp, scale=-1.0)
                nc.gpsimd.dma_start(out=rec_dr[p][:], in_=recb[:])
                rbc = rbcp.tile([128, SL], bf16, tag="rbc")
                rbcv = rbc[:].rearrange("q (k c) -> q k c", k=NQC)
                nc.sync.dma_start(
                    out=rbcv[0:64], in_=rec_dr[p][0:97:32, :].partition_broadcast(64)
                )
                nc.sync.dma_start(
                    out=rbcv[64:128], in_=rec_dr[p][1:98:32, :].partition_broadcast(64)
                )
                if DEBUG:
                    nc.sync.dma_start(out=dbg_rec[:, p, :], in_=rbc[:])
                return rbc

            def emit_num(p, rbc):
                kvbd = bdp.tile([128, 128], bf16, tag="kvbd")
                nc.gpsimd.memset(kvbd[:], 0.0)
                nc.gpsimd.tensor_copy(kvbd[0:64, 0:64], kvrd[0:64, p, 0:64])
                nc.gpsimd.tensor_copy(kvbd[64:128, 64:128], kvrd[64:128, p, 64:128])
                for j in range(NQC):
                    js = slice(j * 512, (j + 1) * 512)
                    nps = ps_num.tile([128, 512], f32, tag="nps")
                    nc.tensor.matmul(
                        nps[:], lhsT=kvbd[:], rhs=qfT[:, p, js],
                        start=True, stop=True,
                    )
                    nc.vector.tensor_tensor(
                        ctxT[:, p, js], nps[:], rbc[:, js], Alu.mult
                    )

            # ---- software-pipelined phase A ----
            kfv = {}
            rbcs = {}
            for p in range(NPAIR):
                kfv[p] = emit_proj(p)
                emit_qf(p)
                if p >= 1:
                    emit_kv(p - 1, *kfv.pop(p - 1))
                if p >= 2:
                    rbcs[p - 2] = emit_den(p - 2)
                if p >= 3:
                    emit_num(p - 3, rbcs.pop(p - 3))
            emit_kv(NPAIR - 1, *kfv.pop(NPAIR - 1))
            rbcs[NPAIR - 2] = emit_den(NPAIR - 2)
            emit_num(NPAIR - 3, rbcs.pop(NPAIR - 3))
            rbcs[NPAIR - 1] = emit_den(NPAIR - 1)
            emit_num(NPAIR - 2, rbcs.pop(NPAIR - 2))
            emit_num(NPAIR - 1, rbcs.pop(NPAIR - 1))
            if DEBUG:
                nc.sync.dma_start(out=dbg_qf[:], in_=qfT[:])
                nc.sync.dma_start(out=dbg_ctx[:], in_=ctxT[:])
                nc.sync.dma_start(out=dbg_kvrd[:], in_=kvrd[:])
            psA.close()

            # ---- phase O: output projection (sequence-major out) ----
            with tc.tile_pool(name="ps_o", bufs=2, space="PSUM") as ps_o:
                for si in range(NCHUNK):
                    ss = slice(si * 128, (si + 1) * 128)
                    ops = ps_o.tile([128, E], f32, tag="ops")
                    for k in range(NPAIR):
                        nc.tensor.matmul(
                            ops[:, 0:512], lhsT=ctxT[:, k, ss],
                            rhs=wo_sb[:, k, 0:512],
                            start=(k == 0), stop=(k == NPAIR - 1),
                        )
                        nc.tensor.matmul(
                            ops[:, 512:E], lhsT=ctxT[:, k, ss],
                            rhs=wo_sb[:, k, 512:E],
                            start=(k == 0), stop=(k == NPAIR - 1),
                        )
                    ysb = outp.tile([128, E], f32, tag="ysb")
                    nc.vector.tensor_copy(ysb[:, 0:512], ops[:, 0:512])
                    nc.scalar.copy(ysb[:, 512:E], ops[:, 512:E])
                    nc.sync.dma_start(out=y[ss, :], in_=ysb[:])

    nc.compile()
    return nc


def _get_program():
    if "nc" not in _CACHE:
        _CACHE["nc"] = _build_program()
    return _CACHE["nc"]


def _host_prep(query, Wq, Wk, Wv, Wo):
    bf16 = ml_dtypes.bfloat16
    q_bf = np.ascontiguousarray(query.astype(bf16))
    wq_bd = np.zeros((NPAIR, 128, 128), dtype=bf16)
    wkv_bd = np.zeros((NPAIR, 128, 256), dtype=bf16)
    for p in range(NPAIR):
        wq_bd[p, 0:64, 0:64] = Wq[2 * p]
        wq_bd[p, 64:128, 64:128] = Wq[2 * p + 1]
        wkv_bd[p, 0:64, 0:64] = Wk[2 * p]
        wkv_bd[p, 64:128, 64:128] = Wk[2 * p + 1]
        wkv_bd[p, 0:64, 128:192] = Wv[2 * p]
        wkv_bd[p, 64:128, 192:256] = Wv[2 * p + 1]
    wo_bf = np.ascontiguousarray(Wo.astype(bf16))
    in_maps = []
    for c in range(N_CORES):
        b, j = divmod(c, 2)
        in_maps.append({
            "xqT": np.ascontiguousarray(q_bf[b, j * SL:(j + 1) * SL, :].T),
            "wq_bd": wq_bd,
            "wkv_bd": wkv_bd,
            "wo": wo_bf,
        })
    return in_maps


def kernel(query, Wq, Wk, Wv, Wo):
    from concourse.bass_utils import run_bass_kernel_spmd

    nc = _get_program()
    in_maps = _host_prep(query, Wq, Wk, Wv, Wo)
    res = run_bass_kernel_spmd(nc, in_maps, list(range(N_CORES)))
    out = np.empty((B, S, E), dtype=np.float32)
    for c in range(N_CORES):
        b, j = divmod(c, 2)
        out[b, j * SL:(j + 1) * SL, :] = res.results[c]["y"]
    return out
